# revision 5
# baseline (speedup 1.0000x reference)
"""EvolveGCN layer on 8 trn2 NeuronCores.

Math: out = relu(segment_sum(h[src] * ew, dst) @ W)   (projection commutes
with the linear aggregation, so we aggregate raw h first and run one GEMM
per 128-dst block afterwards -- no inter-core communication at all).

Sharding: dst nodes are range-partitioned across the 8 cores (12500 each).
Each core gets the full h (gathered from its own DRAM), its edge partition
(sorted by (dst_block, src)), aggregates per 128-dst block via one-hot
scatter matmuls in PSUM, transposes, multiplies by W, applies ReLU.

Gather: dma_gather (Q7 mlp library) with int16 indices wrapped in 16
partitions; h is split into 4 row groups of 25000 so indices fit int16.
"""
import os
import sys

sys.path.insert(0, "/opt/trn_rl_repo")
sys.path.insert(0, "/opt/trn_rl_repo/concourse")

import numpy as np

N_NODES = 100000
N_CORES = 8
D = 512
P = 128
SHARD = N_NODES // N_CORES          # 12500 dst nodes per core
NBLK = (SHARD + P - 1) // P         # 98 dst blocks per core
GROUP = 25000                       # src rows per dma_gather group
NGRP = N_NODES // GROUP             # 4

_LAST_RUN = {}                      # test.py reads exec_time_ns from here


def _host_prep(src, dst, edge_weight):
    """Partition/sort edges; build per-core meta planes.

    Returns dict with per-core arrays and the shared chunk structure.
    """
    src = np.asarray(src).astype(np.int64)
    dst = np.asarray(dst).astype(np.int64)
    ew = np.asarray(edge_weight).astype(np.float32)

    core = dst // SHARD
    per_core = []
    for k in range(N_CORES):
        m = core == k
        s, d, w = src[m], dst[m] - k * SHARD, ew[m]
        blk = d // P
        g = s // GROUP
        order = np.lexsort((s, g, blk))       # sort by (block, group, src)
        s, d, w, blk, g = s[order], d[order], w[order], blk[order], g[order]
        # counts[b, gi] = edges of core k in (block b, group gi)
        counts = np.zeros((NBLK, NGRP), dtype=np.int64)
        np.add.at(counts, (blk, g), 1)
        per_core.append(dict(s=s, d=d, w=w, counts=counts))

    all_counts = np.stack([pc["counts"] for pc in per_core])  # [8, NBLK, NGRP]
    maxc = all_counts.max(axis=0)                             # [NBLK, NGRP]
    cbg = -(-maxc // P)                                       # chunks per (b, g)
    cb = cbg.sum(axis=1)                                      # chunks per block
    tc = int(cb.sum())                                        # total chunks

    # chunk-column base per (b, g)
    base = np.zeros((NBLK, NGRP), dtype=np.int64)
    run = 0
    for b in range(NBLK):
        for gi in range(NGRP):
            base[b, gi] = run
            run += cbg[b, gi]

    metas = []
    for k in range(N_CORES):
        pc = per_core[k]
        dl_plane = np.zeros((P, tc), dtype=np.float16)
        w_plane = np.zeros((P, tc), dtype=np.float16)
        idx_flat = np.full((tc * P,), -1, dtype=np.int16)  # slot-major edge idx
        pos = 0
        for b in range(NBLK):
            for gi in range(NGRP):
                n = int(pc["counts"][b, gi])
                nslot = int(cbg[b, gi]) * P
                if nslot == 0:
                    continue
                sl = slice(pos, pos + n)
                i = np.arange(n)
                c0 = int(base[b, gi])
                lanes = i % P
                cols = c0 + i // P
                dl_plane[lanes, cols] = (pc["d"][sl] - b * P).astype(np.float16)
                w_plane[lanes, cols] = pc["w"][sl].astype(np.float16)
                rel = (pc["s"][sl] - gi * GROUP).astype(np.int16)
                mc = int(maxc[b, gi])
                seg = np.full((nslot,), -1, dtype=np.int16)
                seg[:n] = rel
                seg[n:mc] = 0                  # pad-to-max gathers row 0
                idx_flat[c0 * P:c0 * P + nslot] = seg
                pos += n
        # wrap idxs: position i -> [i % 16, i // 16], replicated to 128 parts
        wrapped = np.zeros((16, tc * P // 16), dtype=np.int16)
        ii = np.arange(tc * P)
        wrapped[ii % 16, ii // 16] = idx_flat
        idx_plane = np.tile(wrapped, (8, 1))
        metas.append(dict(dl=dl_plane, w=w_plane, idx=idx_plane))

    return dict(metas=metas, maxc=maxc, cbg=cbg, cb=cb, base=base, tc=tc)


def _build_program(prep, mm_f32r=True):
    import concourse.bass as bass
    import concourse.mybir as mybir
    import concourse.tile as tile
    from concourse import bacc

    maxc, cbg, cb, base, tc = (
        prep["maxc"], prep["cbg"], prep["cb"], prep["base"], prep["tc"],
    )
    cbmax = int(cb.max())
    f32 = mybir.dt.float32
    f16 = mybir.dt.float16

    nc = bacc.Bacc(None, target_bir_lowering=False, debug=True)
    mmdt = f16
    h_t = nc.declare_dram_parameter("h", [N_NODES, D], mmdt, isOutput=False)
    w_t = nc.declare_dram_parameter("wmat", [D, D], mmdt, isOutput=False)
    dl_t = nc.declare_dram_parameter("dl", [P, tc], f16, isOutput=False)
    ww_t = nc.declare_dram_parameter("ww", [P, tc], f16, isOutput=False)
    ix_t = nc.declare_dram_parameter("ix", [P, tc * 8], mybir.dt.int16, isOutput=False)
    io_t = nc.declare_dram_parameter("iota", [P, P], f16, isOutput=False)
    id_t = nc.declare_dram_parameter("ident", [P, P], f32, isOutput=False)
    out_t = nc.declare_dram_parameter("out", [NBLK * P, D], f32, isOutput=True)


    with tile.TileContext(nc) as tcx:
        with (
            tcx.tile_pool(name="const", bufs=1) as cpool,
            tcx.tile_pool(name="xp", bufs=2) as xp,
            tcx.tile_pool(name="sp", bufs=2) as spool,
            tcx.tile_pool(name="cp", bufs=2) as copies,
            tcx.tile_pool(name="pp", bufs=2, space="PSUM") as pp,
        ):
            dl_s = cpool.tile([P, tc], f16)
            ww_s = cpool.tile([P, tc], f16)
            ix_s = cpool.tile([P, tc * 8], mybir.dt.int16)
            io_s = cpool.tile([P, P], f16)
            id_s = cpool.tile([P, P], f32)
            wm_s = cpool.tile([P, 4, D], mmdt)   # W[j*128+p, o] at [p, j, o]
            nc.sync.dma_start(out=dl_s[:], in_=dl_t[:])
            nc.sync.dma_start(out=ww_s[:], in_=ww_t[:])
            nc.sync.dma_start(out=ix_s[:], in_=ix_t[:])
            nc.sync.dma_start(out=io_s[:], in_=io_t[:])
            nc.sync.dma_start(out=id_s[:], in_=id_t[:])
            nc.sync.dma_start(
                out=wm_s[:],
                in_=w_t[:].rearrange("(a p) o -> p a o", p=P),
            )

            # zero the X slots once: stale tails are masked by w=0 in S, but
            # must be finite
            for _ in range(2):
                xz = xp.tile([P, cbmax, D], mmdt, tag="X")
                nc.vector.memset(xz[:], 0)

            for b in range(NBLK):
                cb_b = int(cb[b])
                if cb_b == 0:
                    continue
                X = xp.tile([P, cbmax, D], mmdt, tag="X")
                for gi in range(NGRP):
                    n_ch = int(cbg[b, gi])
                    if n_ch == 0:
                        continue
                    c0 = int(base[b, gi]) - int(base[b, 0])
                    p0 = int(base[b, gi])  # global chunk col in meta planes
                    nc.gpsimd.dma_gather(
                        out_ap=X[:, c0:c0 + n_ch, :],
                        in_ap=h_t[gi * GROUP:(gi + 1) * GROUP, :],
                        idxs_ap=ix_s[:, p0 * 8:(p0 + n_ch) * 8],
                        num_idxs=n_ch * P,
                        num_idxs_reg=int(maxc[b, gi]),
                        elem_size=D,
                        single_packet=False,
                    )
                agg_ps = pp.tile([P, D], f32, space="PSUM", tag="agg")
                gb = int(base[b, 0])
                S_all = spool.tile([P, cbmax, P], mmdt, tag="S")
                dl_b = bass.AP(
                    dl_s[:].tensor, dl_s[:].offset + gb,
                    [dl_s[:].ap[0], [1, cb_b], [0, P]],
                )
                ww_b = bass.AP(
                    ww_s[:].tensor, ww_s[:].offset + gb,
                    [ww_s[:].ap[0], [1, cb_b], [0, P]],
                )
                io_b = bass.AP(
                    io_s[:].tensor, io_s[:].offset,
                    [io_s[:].ap[0], [0, cb_b], [1, P]],
                )
                nc.vector.tensor_tensor(
                    out=S_all[:, :cb_b, :], in0=io_b, in1=dl_b,
                    op=mybir.AluOpType.is_equal,
                )
                nc.vector.tensor_tensor(
                    out=S_all[:, :cb_b, :], in0=S_all[:, :cb_b, :], in1=ww_b,
                    op=mybir.AluOpType.mult,
                )
                for c in range(cb_b):
                    nc.tensor.matmul(
                        out=agg_ps[:], lhsT=S_all[:, c, :], rhs=X[:, c, :],
                        start=(c == 0), stop=(c == cb_b - 1),
                    )
                agg_sb = copies.tile([P, D], f32, tag="aggsb")
                nc.vector.tensor_copy(out=agg_sb[:], in_=agg_ps[:])
                aggT_ps = pp.tile([P, D], f32, space="PSUM", tag="aggT")
                for j in range(4):
                    nc.tensor.transpose(
                        out=aggT_ps[:, j * P:(j + 1) * P],
                        in_=agg_sb[:, j * P:(j + 1) * P],
                        identity=id_s[:],
                    )
                aggT_sb = copies.tile([P, D], mmdt, tag="aggTsb")
                nc.vector.tensor_copy(out=aggT_sb[:], in_=aggT_ps[:])
                out_ps = pp.tile([P, D], f32, space="PSUM", tag="out")
                for j in range(4):
                    nc.tensor.matmul(
                        out=out_ps[:],
                        lhsT=aggT_sb[:, j * P:(j + 1) * P],
                        rhs=wm_s[:, j, :],
                        start=(j == 0), stop=(j == 3),
                    )
                out_sb = copies.tile([P, D], f32, tag="outsb")
                nc.scalar.activation(
                    out_sb[:], out_ps[:], mybir.ActivationFunctionType.Relu
                )
                nc.sync.dma_start(
                    out=out_t[b * P:(b + 1) * P, :], in_=out_sb[:]
                )
    nc.compile()
    return nc


def kernel(h, weight, edge_weight, src, dst):
    from concourse.bass_utils import run_bass_kernel_spmd

    h = np.ascontiguousarray(np.asarray(h), dtype=np.float16)
    weight = np.ascontiguousarray(np.asarray(weight), dtype=np.float16)

    prep = _host_prep(src, dst, edge_weight)
    nc = _build_program(prep, mm_f32r=os.environ.get("KERNEL_FP32", "0") != "1")

    iota = np.broadcast_to(
        np.arange(P, dtype=np.float16)[None, :], (P, P)
    ).copy()
    ident = np.eye(P, dtype=np.float32)
    in_maps = []
    for k in range(N_CORES):
        m = prep["metas"][k]
        in_maps.append({
            "h": h, "wmat": weight, "dl": m["dl"], "ww": m["w"],
            "ix": m["idx"], "iota": iota, "ident": ident,
        })

    trace = os.environ.get("KERNEL_TRACE", "0") == "1"
    kw = {}
    if trace:
        kw = dict(trace=True)
    res = run_bass_kernel_spmd(nc, in_maps, core_ids=list(range(N_CORES)), **kw)
    _LAST_RUN["exec_time_ns"] = res.exec_time_ns
    _LAST_RUN["results"] = res

    out = np.empty((N_NODES, D), dtype=np.float32)
    for k in range(N_CORES):
        out[k * SHARD:(k + 1) * SHARD] = res.results[k]["out"][:SHARD]
    return out


# revision 6
# speedup vs baseline: 1.1080x; 1.1080x over previous
"""EvolveGCN layer on 8 trn2 NeuronCores.

Math: out = relu(segment_sum(h[src] * ew, dst) @ W)   (projection commutes
with the linear aggregation, so we aggregate raw h first and run one GEMM
per 128-dst block afterwards -- no inter-core communication at all).

Sharding: dst nodes are range-partitioned across the 8 cores (12500 each).
Each core gets the full h (gathered from its own DRAM), its edge partition
(sorted by (block_pair, src_group, block, src)), aggregates per 128-dst
block via one-hot scatter matmuls in PSUM, transposes, multiplies by W,
applies ReLU.

Gather: dma_gather (Q7 mlp library) with int16 indices wrapped in 16
partitions; h is split into 4 row groups of 25000 so indices fit int16.
One gather per (block_pair, group) covers ~1150 rows.
"""
import os
import sys

sys.path.insert(0, "/opt/trn_rl_repo")
sys.path.insert(0, "/opt/trn_rl_repo/concourse")

import numpy as np

N_NODES = 100000
N_CORES = 8
D = 512
P = 128
SHARD = N_NODES // N_CORES          # 12500 dst nodes per core
NBLK = (SHARD + P - 1) // P         # 98 dst blocks per core
NPAIR = NBLK // 2                   # 49 block pairs
GROUP = 25000                       # src rows per dma_gather group
NGRP = N_NODES // GROUP             # 4

_LAST_RUN = {}                      # test.py reads exec_time_ns from here


def _host_prep(src, dst, edge_weight):
    src = np.asarray(src).astype(np.int64)
    dst = np.asarray(dst).astype(np.int64)
    ew = np.asarray(edge_weight).astype(np.float32)

    core = dst // SHARD
    per_core = []
    for k in range(N_CORES):
        m = core == k
        s, d, w = src[m], dst[m] - k * SHARD, ew[m]
        blk = d // P
        g = s // GROUP
        pb = blk // 2
        order = np.lexsort((s, blk, g, pb))   # (pair, group, block, src)
        s, d, w, blk, g = s[order], d[order], w[order], blk[order], g[order]
        counts = np.zeros((NBLK, NGRP), dtype=np.int64)
        np.add.at(counts, (blk, g), 1)
        per_core.append(dict(s=s, d=d, w=w, counts=counts))

    all_counts = np.stack([pc["counts"] for pc in per_core])  # [8, NBLK, NGRP]
    maxc = all_counts.max(axis=0)                             # [NBLK, NGRP]
    cbg = -(-maxc // P)                                       # chunks per (b, g)

    # chunk-column layout: for pb, for g: [b0 chunks][b1 chunks]
    col0 = np.zeros((NBLK, NGRP), dtype=np.int64)
    pairbase = np.zeros(NPAIR + 1, dtype=np.int64)
    run = 0
    for pb in range(NPAIR):
        pairbase[pb] = run
        for gi in range(NGRP):
            col0[2 * pb, gi] = run
            run += cbg[2 * pb, gi]
            col0[2 * pb + 1, gi] = run
            run += cbg[2 * pb + 1, gi]
    pairbase[NPAIR] = run
    tc = int(run)

    # per-(pb, g) gather: start col, n chunks, num_idxs_reg
    g_start = np.zeros((NPAIR, NGRP), dtype=np.int64)
    g_nch = np.zeros((NPAIR, NGRP), dtype=np.int64)
    g_reg = np.zeros((NPAIR, NGRP), dtype=np.int64)
    for pb in range(NPAIR):
        for gi in range(NGRP):
            b0, b1 = 2 * pb, 2 * pb + 1
            g_start[pb, gi] = col0[b0, gi]
            g_nch[pb, gi] = cbg[b0, gi] + cbg[b1, gi]
            g_reg[pb, gi] = cbg[b0, gi] * P + maxc[b1, gi]

    metas = []
    for k in range(N_CORES):
        pc = per_core[k]
        dl_plane = np.zeros((P, tc), dtype=np.float16)
        w_plane = np.zeros((P, tc), dtype=np.float16)
        idx_flat = np.full((tc * P,), -1, dtype=np.int16)
        pos = 0
        for pb in range(NPAIR):
            for gi in range(NGRP):
                for half, b in enumerate((2 * pb, 2 * pb + 1)):
                    n = int(pc["counts"][b, gi])
                    nslot = int(cbg[b, gi]) * P
                    if nslot == 0:
                        continue
                    sl = slice(pos, pos + n)
                    i = np.arange(n)
                    c0 = int(col0[b, gi])
                    lanes, cols = i % P, c0 + i // P
                    dl_plane[lanes, cols] = (pc["d"][sl] - b * P).astype(np.float16)
                    w_plane[lanes, cols] = pc["w"][sl].astype(np.float16)
                    rel = (pc["s"][sl] - gi * GROUP).astype(np.int16)
                    if half == 0:
                        # first block of the pair: fully gathered (pad 0)
                        seg = np.zeros((nslot,), dtype=np.int16)
                        seg[:n] = rel
                    else:
                        mc = int(maxc[b, gi])
                        seg = np.full((nslot,), -1, dtype=np.int16)
                        seg[:n] = rel
                        seg[n:mc] = 0
                    idx_flat[c0 * P:c0 * P + nslot] = seg
                    pos += n
        wrapped = np.zeros((16, tc * P // 16), dtype=np.int16)
        ii = np.arange(tc * P)
        wrapped[ii % 16, ii // 16] = idx_flat
        idx_plane = np.tile(wrapped, (8, 1))
        metas.append(dict(dl=dl_plane, w=w_plane, idx=idx_plane))

    return dict(metas=metas, cbg=cbg, col0=col0, pairbase=pairbase, tc=tc,
                g_start=g_start, g_nch=g_nch, g_reg=g_reg)


def _build_program(prep):
    import concourse.bass as bass
    import concourse.mybir as mybir
    import concourse.tile as tile
    from concourse import bacc

    cbg, col0, pairbase, tc = (
        prep["cbg"], prep["col0"], prep["pairbase"], prep["tc"],
    )
    g_start, g_nch, g_reg = prep["g_start"], prep["g_nch"], prep["g_reg"]
    cbp = pairbase[1:] - pairbase[:-1]          # chunks per pair
    cbpmax = int(cbp.max())
    f32 = mybir.dt.float32
    f16 = mybir.dt.float16

    nc = bacc.Bacc(None, target_bir_lowering=False, debug=True)
    h_t = nc.declare_dram_parameter("h", [N_NODES, D], f16, isOutput=False)
    w_t = nc.declare_dram_parameter("wmat", [D, D], f16, isOutput=False)
    dl_t = nc.declare_dram_parameter("dl", [P, tc], f16, isOutput=False)
    ww_t = nc.declare_dram_parameter("ww", [P, tc], f16, isOutput=False)
    ix_t = nc.declare_dram_parameter("ix", [P, tc * 8], mybir.dt.int16, isOutput=False)
    io_t = nc.declare_dram_parameter("iota", [P, P], f16, isOutput=False)
    id_t = nc.declare_dram_parameter("ident", [P, P], f32, isOutput=False)
    out_t = nc.declare_dram_parameter("out", [NBLK * P, D], f32, isOutput=True)

    with tile.TileContext(nc) as tcx:
        with (
            tcx.tile_pool(name="const", bufs=1) as cpool,
            tcx.tile_pool(name="xp", bufs=2) as xp,
            tcx.tile_pool(name="sp", bufs=2) as spool,
            tcx.tile_pool(name="cp", bufs=2) as copies,
            tcx.tile_pool(name="pp", bufs=2, space="PSUM") as pp,
        ):
            dl_s = cpool.tile([P, tc], f16)
            ww_s = cpool.tile([P, tc], f16)
            ix_s = cpool.tile([P, tc * 8], mybir.dt.int16)
            io_s = cpool.tile([P, P], f16)
            id_s = cpool.tile([P, P], f32)
            wm_s = cpool.tile([P, 4, D], f16)   # W[j*128+p, o] at [p, j, o]
            nc.sync.dma_start(out=dl_s[:], in_=dl_t[:])
            nc.sync.dma_start(out=ww_s[:], in_=ww_t[:])
            nc.sync.dma_start(out=ix_s[:], in_=ix_t[:])
            nc.sync.dma_start(out=io_s[:], in_=io_t[:])
            nc.sync.dma_start(out=id_s[:], in_=id_t[:])
            nc.sync.dma_start(
                out=wm_s[:], in_=w_t[:].rearrange("(a p) o -> p a o", p=P),
            )

            # zero X slots once (stale tails are w=0-masked; must be finite)
            for _ in range(2):
                xz = xp.tile([P, cbpmax, D], f16, tag="X")
                nc.vector.memset(xz[:], 0)

            for pb in range(NPAIR):
                pb0 = int(pairbase[pb])
                n_pair = int(cbp[pb])
                if n_pair == 0:
                    continue
                X = xp.tile([P, cbpmax, D], f16, tag="X")
                for gi in range(NGRP):
                    n_ch = int(g_nch[pb, gi])
                    if n_ch == 0:
                        continue
                    c0 = int(g_start[pb, gi]) - pb0
                    p0 = int(g_start[pb, gi])
                    nc.gpsimd.dma_gather(
                        out_ap=X[:, c0:c0 + n_ch, :],
                        in_ap=h_t[gi * GROUP:(gi + 1) * GROUP, :],
                        idxs_ap=ix_s[:, p0 * 8:(p0 + n_ch) * 8],
                        num_idxs=n_ch * P,
                        num_idxs_reg=int(g_reg[pb, gi]),
                        elem_size=D,
                        single_packet=False,
                    )
                # S for the whole pair in two broadcast DVE ops
                S_all = spool.tile([P, cbpmax, P], f16, tag="S")
                dl_ap = dl_s[:]
                dl_b = bass.AP(dl_ap.tensor, dl_ap.offset + pb0,
                               [dl_ap.ap[0], [1, n_pair], [0, P]])
                ww_ap = ww_s[:]
                ww_b = bass.AP(ww_ap.tensor, ww_ap.offset + pb0,
                               [ww_ap.ap[0], [1, n_pair], [0, P]])
                io_ap = io_s[:]
                io_b = bass.AP(io_ap.tensor, io_ap.offset,
                               [io_ap.ap[0], [0, n_pair], [1, P]])
                nc.vector.tensor_tensor(
                    out=S_all[:, :n_pair, :], in0=io_b, in1=dl_b,
                    op=mybir.AluOpType.is_equal,
                )
                nc.vector.tensor_tensor(
                    out=S_all[:, :n_pair, :], in0=S_all[:, :n_pair, :], in1=ww_b,
                    op=mybir.AluOpType.mult,
                )

                for b in (2 * pb, 2 * pb + 1):
                    cols = []
                    for gi in range(NGRP):
                        s0 = int(col0[b, gi]) - pb0
                        cols.extend(range(s0, s0 + int(cbg[b, gi])))
                    if not cols:
                        continue
                    agg_ps = pp.tile([P, D], f32, space="PSUM", tag="agg")
                    for ci, c in enumerate(cols):
                        nc.tensor.matmul(
                            out=agg_ps[:], lhsT=S_all[:, c, :], rhs=X[:, c, :],
                            start=(ci == 0), stop=(ci == len(cols) - 1),
                        )
                    agg_sb = copies.tile([P, D], f32, tag="aggsb")
                    nc.vector.tensor_copy(out=agg_sb[:], in_=agg_ps[:])
                    aggT_ps = pp.tile([P, D], f32, space="PSUM", tag="aggT")
                    for j in range(4):
                        nc.tensor.transpose(
                            out=aggT_ps[:, j * P:(j + 1) * P],
                            in_=agg_sb[:, j * P:(j + 1) * P],
                            identity=id_s[:],
                        )
                    aggT_sb = copies.tile([P, D], f16, tag="aggTsb")
                    nc.vector.tensor_copy(out=aggT_sb[:], in_=aggT_ps[:])
                    out_ps = pp.tile([P, D], f32, space="PSUM", tag="out")
                    for j in range(4):
                        nc.tensor.matmul(
                            out=out_ps[:],
                            lhsT=aggT_sb[:, j * P:(j + 1) * P],
                            rhs=wm_s[:, j, :],
                            start=(j == 0), stop=(j == 3),
                        )
                    out_sb = copies.tile([P, D], f32, tag="outsb")
                    nc.scalar.activation(
                        out_sb[:], out_ps[:], mybir.ActivationFunctionType.Relu
                    )
                    nc.sync.dma_start(
                        out=out_t[b * P:(b + 1) * P, :], in_=out_sb[:]
                    )
    nc.compile()
    return nc


def kernel(h, weight, edge_weight, src, dst):
    from concourse.bass_utils import run_bass_kernel_spmd

    h = np.ascontiguousarray(np.asarray(h), dtype=np.float16)
    weight = np.ascontiguousarray(np.asarray(weight), dtype=np.float16)

    prep = _host_prep(src, dst, edge_weight)
    nc = _build_program(prep)

    iota = np.broadcast_to(
        np.arange(P, dtype=np.float16)[None, :], (P, P)
    ).copy()
    ident = np.eye(P, dtype=np.float32)
    in_maps = []
    for k in range(N_CORES):
        m = prep["metas"][k]
        in_maps.append({
            "h": h, "wmat": weight, "dl": m["dl"], "ww": m["w"],
            "ix": m["idx"], "iota": iota, "ident": ident,
        })

    trace = os.environ.get("KERNEL_TRACE", "0") == "1"
    kw = dict(trace=True) if trace else {}
    res = run_bass_kernel_spmd(nc, in_maps, core_ids=list(range(N_CORES)), **kw)
    _LAST_RUN["exec_time_ns"] = res.exec_time_ns
    _LAST_RUN["results"] = res

    out = np.empty((N_NODES, D), dtype=np.float32)
    for k in range(N_CORES):
        out[k * SHARD:(k + 1) * SHARD] = res.results[k]["out"][:SHARD]
    return out
